# revision 4
# baseline (speedup 1.0000x reference)
"""BorderLoss Trainium2 kernel (v5).

Reference (per element, then global mean over [64,512,512]):
    loss = softplus((1-2y)*x)   (stable BCE identity, y binary)
    m = (y > 0);  border = dilate3x3(m) - erode3x3(m)  (SAME, OOB ignored)
    w = 1 + border;  out = mean(loss * w)

Scheme (v5 -- validated elementwise-exact vs reference in numpy):
  * Work on m2 = m - 0.5 (in {-0.5,+0.5}), delivered DIRECTLY by the DMA:
    memset the fp8 tile to -0.5 (zero pads), then SWDGE cast-DMA y with
    accum_op=add.  Saves the per-image DVE tensor_scalar of v4.
  * v2 = 3x3 weighted box-sum of m2 (OOB=0; image rows 0/511 use a 1.5x
    2-row vertical scale).  Shift algebra: v = v2 + 4.5 everywhere except
    edge COLUMNS where v = v2 + 3, so ONE uniform band test |v2| <= 4.05
    is exact except at columns 0/511, fixed by subtracting l where
    v2 >= 2.5 or v2 <= -2.5 on those columns (two strided PSUM STTs;
    corners exact).
  * Horizontal 3-tap: outer pair via one DVE fp8 add into the t-section
    of a [P, 2, FI] tile; the center tap is folded via fp8 DoubleRow
    matmuls: each vertical pass computes W^T@t2 + W^T@m2c in ONE PE pass
    (2x throughput), rhs = [P, 2, 512] AP spanning the (t, m) sections.
  * Vertical 3-tap: per 128-row block, tridiagonal DoubleRow matmul with
    single-entry U/L matrices carrying the cross-block rows.  10 matmuls
    per image (4 main + 3 U + 3 L) into one [P, 4*512] PSUM tile.
  * loss: zh = m2 * x (DVE TT, x kept f32 from a raw HWDGE load -- TT is
    1x-rate regardless), then ACT Exp(scale=-2) and Ln(bias=1) =
    softplus((1-2m)x) with accum_out giving sum(l) free.  (ACT Softplus
    is broken in this toolchain -- table slot 'act2' mismatch.)
  * border-weighted sum, alternating per image to balance ACT vs DVE:
    even images: ACT Abs(v2) over all 4 blocks + one DVE STT
    (<=4.05)*l; odd images: Abs over blocks 0-2 + STT, block 3 via two
    one-sided PSUM STTs (>=-4.05 minus >=4.05)*l.  Host combine:
      total = sum(l) + band terms - column fixes
"""

import sys
import numpy as np

if "/opt/trn_rl_repo" not in sys.path:
    sys.path.insert(0, "/opt/trn_rl_repo")

# ---- pin exp/ln/abs to the single covering activation-table set ----
from concourse import hw_specs as _hw
import functools as _ft

if not getattr(_hw.get_activation_tables, "_borderloss_patched", False):
    _orig_tabs = getattr(_hw.get_activation_tables, "__wrapped__",
                         _hw.get_activation_tables)

    @_ft.cache
    def _patched_tabs(module_arch):
        from concourse import mybir as _mb
        A = _mb.ActivationFunctionType
        strip = {A.Exp, A.Ln, A.Abs, A.Square}
        out = {}
        for k, v in _orig_tabs(module_arch).items():
            out[k] = v if k == "natural_log_exp_and_others" else v - strip
        return out

    _patched_tabs._borderloss_patched = True
    _hw.get_activation_tables = _patched_tabs

H = W = 512
P = 128
NB = 4               # 128-row blocks per image
FB = 514             # padded block width (data at cols 1..512, zeros at 0, 513)
FI = NB * FB         # 2056 padded free cols per image
FD = NB * W          # 2048 dense free cols per image
NACC = 6
N_CORES = 8

_CACHE = {}


def _consts():
    import ml_dtypes
    f8 = ml_dtypes.float8_e4m3
    tri = np.zeros((P, P), dtype=np.float64)
    for k in range(P):
        tri[k, max(0, k - 1):min(P, k + 2)] = 1.0
    t0 = tri.copy()
    t0[:, 0] = 0.0
    t0[0:2, 0] = 1.5          # image row 0: 1.5x 2-row scale
    t3 = tri.copy()
    t3[:, 127] = 0.0
    t3[126:128, 127] = 1.5    # image row 511
    u = np.zeros((P, P), dtype=np.float64)
    u[0, 127] = 1.0           # next block's row 0 -> out row 127
    lm = np.zeros((P, P), dtype=np.float64)
    lm[127, 0] = 1.0          # prev block's row 127 -> out row 0
    # DoubleRow: each weight doubled as two consecutive [P,P] blocks (A|B),
    # A for the t-section pass, B for the m-section pass; A == B here.
    wts = np.concatenate([w for m in (t0, tri, t3, u, lm) for w in (m, m)],
                         axis=1).astype(f8)
    return wts


def _build(n_imgs):
    import concourse.bass as bass
    import concourse.bacc as bacc
    import concourse.tile as tile
    from concourse import mybir

    f32 = mybir.dt.float32
    bf16 = mybir.dt.bfloat16
    f8 = mybir.dt.float8e4
    i32 = mybir.dt.int32
    Alu = mybir.AluOpType
    Act = mybir.ActivationFunctionType
    MM = mybir.MatmulPerfMode

    nc = bacc.Bacc(None, target_bir_lowering=False)
    x_d = nc.dram_tensor("x", [n_imgs, H, W], f32, kind="ExternalInput")
    y_d = nc.dram_tensor("y", [n_imgs, H, W], i32, kind="ExternalInput")
    w_d = nc.dram_tensor("wts", [P, 10 * P], f8, kind="ExternalInput")
    acc_d = nc.dram_tensor("acc", [P, n_imgs * NACC], f32, kind="ExternalOutput")

    with tile.TileContext(nc) as tc:
        with (
            tc.tile_pool(name="consts", bufs=1) as cpool,
            tc.tile_pool(name="inputs", bufs=1) as ipool,
            tc.tile_pool(name="work", bufs=3) as work,
            tc.tile_pool(name="accp", bufs=1) as apool,
            tc.tile_pool(name="ps", bufs=2, space=bass.MemorySpace.PSUM) as pp,
        ):
            wts = cpool.tile([P, 10 * P], f8)
            nc.sync.dma_start(wts[:], w_d[:])
            wq = wts.rearrange("p (w two m) -> p w two m", w=5, two=2)
            W_T0 = wq[:, 0]
            W_TRI = wq[:, 1]
            W_T3 = wq[:, 2]
            W_U = wq[:, 3]
            W_L = wq[:, 4]

            tms, xs, accs = [], [], []
            for i in range(n_imgs):
                tm = ipool.tile([P, 2, FI], f8, tag=f"tm{i}", name=f"tm{i}")
                tms.append(tm)
                xs.append(ipool.tile([P, FD], f32, tag=f"x{i}", name=f"x{i}"))
                accs.append(apool.tile([P, NACC], f32, tag=f"a{i}", name=f"a{i}"))

            # prefetch: x on the sync HWDGE queue; y on the gpsimd SWDGE
            # queue as a cast-DMA accumulating into the -0.5-memset tile
            for i in range(n_imgs):
                nc.sync.dma_start(
                    xs[i].rearrange("p (b c) -> p b c", c=W),
                    x_d[i].rearrange("(b p) w -> p b w", p=P))
                m2s = tms[i][:, 1]                      # m-section [P, FI]
                m2b = m2s.rearrange("p (b c) -> p b c", c=FB)
                nc.gpsimd.memset(m2s[:], -0.5)
                nc.gpsimd.memset(m2b[:, :, 0:FB:FB - 1], 0)   # pads
                nc.gpsimd.dma_start(
                    m2b[:, :, 1:FB - 1],
                    y_d[i].rearrange("(b p) w -> p b w", p=P),
                    accum_op=Alu.add)

            # PE warm-up while the first loads land
            warm = pp.tile([P, NB, W], f32, tag="sp", name="warm")
            wfull = wts.rearrange("p (two m) -> p two m", two=2)
            for r in range(10):
                nc.tensor.matmul(warm[:, 0], W_TRI, wfull[:, :, 0:W],
                                 start=True, stop=True,
                                 perf_mode=MM.DoubleRow)

            def frontA(i):
                """t-add (fp8) and the vertical DoubleRow matmuls."""
                tm = tms[i]
                t2s = tm[:, 0]                          # t-section [P, FI]
                m2s = tm[:, 1]
                nc.vector.tensor_add(t2s[:, 1:FI - 1], m2s[:, 0:FI - 2],
                                     m2s[:, 2:FI])
                tm3 = tm.rearrange("p two (b c) -> p two b c", c=FB)

                sp = pp.tile([P, NB, W], f32, tag="sp", name=f"sp{i}")

                def rhs(b):
                    return tm3[:, :, b, 1:FB - 1]

                nc.tensor.matmul(sp[:, 0], W_T0, rhs(0), start=True,
                                 stop=False, perf_mode=MM.DoubleRow)
                nc.tensor.matmul(sp[:, 1], W_TRI, rhs(1), start=True,
                                 stop=False, perf_mode=MM.DoubleRow)
                nc.tensor.matmul(sp[:, 2], W_TRI, rhs(2), start=True,
                                 stop=False, perf_mode=MM.DoubleRow)
                nc.tensor.matmul(sp[:, 3], W_T3, rhs(3), start=True,
                                 stop=False, perf_mode=MM.DoubleRow)
                nc.tensor.matmul(sp[:, 0], W_U, rhs(1), start=False,
                                 stop=True, perf_mode=MM.DoubleRow)
                nc.tensor.matmul(sp[:, 1], W_U, rhs(2), start=False,
                                 stop=False, perf_mode=MM.DoubleRow)
                nc.tensor.matmul(sp[:, 2], W_U, rhs(3), start=False,
                                 stop=False, perf_mode=MM.DoubleRow)
                nc.tensor.matmul(sp[:, 1], W_L, rhs(0), start=False,
                                 stop=True, perf_mode=MM.DoubleRow)
                nc.tensor.matmul(sp[:, 2], W_L, rhs(1), start=False,
                                 stop=True, perf_mode=MM.DoubleRow)
                nc.tensor.matmul(sp[:, 3], W_L, rhs(2), start=False,
                                 stop=True, perf_mode=MM.DoubleRow)
                return sp

            def frontB(i):
                """zh = m2*x on DVE, exp/ln softplus on ACT."""
                tm, xb, ac = tms[i], xs[i], accs[i]
                tm3 = tm.rearrange("p two (b c) -> p two b c", c=FB)
                mc = tm3[:, 1, :, 1:FB - 1]             # [P, 4, 512]

                zh = work.tile([P, FD], bf16, tag="zh", name=f"zh{i}")
                nc.vector.tensor_mul(
                    zh.rearrange("p (b c) -> p b c", c=W), mc,
                    xb.rearrange("p (b c) -> p b c", c=W))
                eb = work.tile([P, FD], bf16, tag="eb", name=f"eb{i}")
                nc.scalar.activation(eb[:], zh[:], Act.Exp, scale=-2.0)
                lt = work.tile([P, FD], bf16, tag="lt", name=f"lt{i}")
                nc.scalar.activation(lt[:], eb[:], Act.Ln, bias=1.0,
                                     accum_out=ac[:, 0:1])
                return lt

            def back(i, sp, lt):
                """abs, band STTs, column fixes, accumulator DMA-out.

                Alternates the abs-path block count (4 vs 3) per image to
                balance ACT vs DVE load.
                """
                ac = accs[i]
                ab_blocks = 4 if i % 2 == 0 else 3
                fa = ab_blocks * W
                spd = sp.rearrange("p b c -> p (b c)")
                lt3 = lt.rearrange("p (b c) -> p b c", c=W)

                ab = work.tile([P, FD], bf16, tag="ab", name=f"ab{i}")
                nc.scalar.activation(ab[:, 0:fa], spd[:, 0:fa], Act.Abs)
                u1 = work.tile([P, FD], bf16, tag="u1", name=f"u1{i}")
                nc.vector.scalar_tensor_tensor(
                    u1[:, 0:fa], ab[:, 0:fa], 4.05, lt[:, 0:fa],
                    Alu.is_le, Alu.mult, accum_out=ac[:, 1:2])

                if ab_blocks == 3:
                    u2 = work.tile([P, W], bf16, tag="u2", name=f"u2{i}")
                    nc.vector.scalar_tensor_tensor(
                        u2[:], sp[:, 3], -4.05, lt3[:, 3],
                        Alu.is_ge, Alu.mult, accum_out=ac[:, 2:3])
                    u3 = work.tile([P, W], bf16, tag="u3", name=f"u3{i}")
                    nc.vector.scalar_tensor_tensor(
                        u3[:], sp[:, 3], 4.05, lt3[:, 3],
                        Alu.is_ge, Alu.mult, accum_out=ac[:, 3:4])
                else:
                    nc.vector.memset(ac[:, 2:4], 0.0)

                e1 = work.tile([P, 2 * NB], bf16, tag="e1", name=f"e1{i}")
                nc.vector.scalar_tensor_tensor(
                    e1.rearrange("p (b c) -> p b c", c=2),
                    sp[:, :, ::W - 1], 2.5, lt3[:, :, ::W - 1],
                    Alu.is_ge, Alu.mult, accum_out=ac[:, 4:5])
                e2 = work.tile([P, 2 * NB], bf16, tag="e2", name=f"e2{i}")
                nc.vector.scalar_tensor_tensor(
                    e2.rearrange("p (b c) -> p b c", c=2),
                    sp[:, :, ::W - 1], -2.5, lt3[:, :, ::W - 1],
                    Alu.is_le, Alu.mult, accum_out=ac[:, 5:6])

                nc.sync.dma_start(acc_d[:, i * NACC:(i + 1) * NACC], ac[:])

            sps, lts = {}, {}
            sps[0] = frontA(0)
            lts[0] = frontB(0)
            for i in range(1, n_imgs):
                sps[i] = frontA(i)
                back(i - 1, sps[i - 1], lts[i - 1])
                lts[i] = frontB(i)
            back(n_imgs - 1, sps[n_imgs - 1], lts[n_imgs - 1])

    nc.compile()
    return nc


def _get_nc(n_imgs):
    if n_imgs not in _CACHE:
        _CACHE[n_imgs] = _build(n_imgs)
    return _CACHE[n_imgs]


def _combine(acc, n_imgs):
    a = acc.reshape(P, n_imgs, NACC).astype(np.float64)
    # total = sum(l) + u1 + u2 - u3 - e1 - e2
    return (a[:, :, 0].sum() + a[:, :, 1].sum() + a[:, :, 2].sum()
            - a[:, :, 3].sum() - a[:, :, 4].sum() - a[:, :, 5].sum())


def kernel(x, y):
    from concourse import bass_utils

    n = x.shape[0]
    per = n // N_CORES
    nc = _get_nc(per)
    wts = _consts()
    x = np.ascontiguousarray(x, dtype=np.float32)
    y = np.ascontiguousarray(y, dtype=np.int32)
    in_maps = [
        {"x": x[c * per:(c + 1) * per], "y": y[c * per:(c + 1) * per],
         "wts": wts}
        for c in range(N_CORES)
    ]
    res = bass_utils.run_bass_kernel_spmd(nc, in_maps, core_ids=list(range(N_CORES)))
    total = 0.0
    for r in res.results:
        total += _combine(r["acc"], per)
    return np.float32(total / (n * H * W))


# revision 5
# speedup vs baseline: 1.0867x; 1.0867x over previous
"""BorderLoss Trainium2 kernel (v6 = v4 + select-path rebalance).

Reference (per element, then global mean over [64,512,512]):
    loss = softplus((1-2y)*x)   (stable BCE identity, y binary)
    m = (y > 0);  border = dilate3x3(m) - erode3x3(m)  (SAME, OOB ignored)
    w = 1 + border;  out = mean(loss * w)

Scheme (v4 core, validated elementwise-exact vs reference in numpy):
  * v = 3x3 box-count of m with OOB=0, computed as horizontal 3-tap then
    vertical 3-tap.  Rows 0/511 get an extra 1.5x scale (folded into the
    tridiagonal matmul weights), after which ONE uniform band test
    |v - 4.5| <= 4.05  (i.e. 1 <= v <= 8) is exact everywhere except
    columns 0/511, fixed by a single strided STT with threshold 5.5
    (which also handles the corners exactly).
  * Horizontal 3-tap: outer pair (left+right) via one DVE bf16 2x add on
    a padded layout [P, 4, 516] (pads zero); the center tap is folded
    into the vertical matmul by running every tridiag/U/L pass twice.
  * Vertical 3-tap: per 128-row block, tridiagonal matmul on PE with
    single-entry U/L matrices carrying the cross-block rows.  All 20
    matmuls per image land in ONE [P, 4, 512] PSUM tile (v6).
  * loss: zh = (m - 0.5) * x  (DVE TS+TT), then ACT Exp(scale=-2) and
    Ln(bias=1) = softplus((1-2m)x), with accum_out giving sum(l) free.
  * border-weighted sum (v6 rebalance): ALTERNATING per image --
    even images: ACT Abs(v-4.5) over all 4 blocks + one DVE STT
    (<=4.05)*l;  odd images: Abs over blocks 0-2 + STT, block 3 via two
    one-sided PSUM STTs (>=0.45 minus >=8.55)*l.  This equalizes ACT vs
    DVE load (v4 pinned 3/1 and left DVE the bottleneck).  One merged
    column-fix STT (>=5.5 at cols 0/511 of all 4 blocks) replaces v4's
    two.  Host combine:
      total = sum(l) + u1 + u2 - u3 - colfix
"""

import sys
import numpy as np

if "/opt/trn_rl_repo" not in sys.path:
    sys.path.insert(0, "/opt/trn_rl_repo")

# ---- pin exp/ln/abs/square to the single covering activation-table set ----
from concourse import hw_specs as _hw
import functools as _ft

if not getattr(_hw.get_activation_tables, "_borderloss_patched", False):
    _orig_tabs = getattr(_hw.get_activation_tables, "__wrapped__",
                         _hw.get_activation_tables)

    @_ft.cache
    def _patched_tabs(module_arch):
        from concourse import mybir as _mb
        A = _mb.ActivationFunctionType
        strip = {A.Exp, A.Ln, A.Abs, A.Square}
        out = {}
        for k, v in _orig_tabs(module_arch).items():
            out[k] = v if k == "natural_log_exp_and_others" else v - strip
        return out

    _patched_tabs._borderloss_patched = True
    _hw.get_activation_tables = _patched_tabs

H = W = 512
P = 128
NB = 4               # 128-row blocks per image
FB = 516             # padded block width (data at cols 2..513, zeros at 1, 514)
FI = NB * FB         # 2064 padded free cols per image
FD = NB * W          # 2048 dense free cols per image
NACC = 5
N_CORES = 8

_CACHE = {}


def _consts():
    import ml_dtypes
    bf = ml_dtypes.bfloat16
    tri = np.zeros((P, P), dtype=np.float64)
    for k in range(P):
        tri[k, max(0, k - 1):min(P, k + 2)] = 1.0
    t0 = tri.copy()
    t0[0:2, 0] = 1.5          # scale row 0 so the uniform band is exact
    t3 = tri.copy()
    t3[126:128, 127] = 1.5
    u = np.zeros((P, P), dtype=np.float64)
    u[0, 127] = 1.0           # next block's row 0 -> out row 127
    lm = np.zeros((P, P), dtype=np.float64)
    lm[127, 0] = 1.0          # prev block's row 127 -> out row 0
    wts = np.concatenate([t0, tri, t3, u, lm], axis=1).astype(bf)
    return wts


def _build(n_imgs):
    import concourse.bass as bass
    import concourse.bacc as bacc
    import concourse.tile as tile
    from concourse import mybir

    f32 = mybir.dt.float32
    bf16 = mybir.dt.bfloat16
    i32 = mybir.dt.int32
    Alu = mybir.AluOpType
    Act = mybir.ActivationFunctionType

    nc = bacc.Bacc(None, target_bir_lowering=False)
    x_d = nc.dram_tensor("x", [n_imgs, H, W], f32, kind="ExternalInput")
    y_d = nc.dram_tensor("y", [n_imgs, H, W], i32, kind="ExternalInput")
    w_d = nc.dram_tensor("wts", [P, 5 * P], bf16, kind="ExternalInput")
    acc_d = nc.dram_tensor("acc", [P, n_imgs * NACC], f32, kind="ExternalOutput")

    with tile.TileContext(nc) as tc:
        with (
            tc.tile_pool(name="consts", bufs=1) as cpool,
            tc.tile_pool(name="inputs", bufs=1) as ipool,
            tc.tile_pool(name="work", bufs=4) as work,
            tc.tile_pool(name="accp", bufs=1) as apool,
            tc.tile_pool(name="ps", bufs=2, space=bass.MemorySpace.PSUM) as pp,
        ):
            wts = cpool.tile([P, 5 * P], bf16)
            nc.sync.dma_start(wts[:], w_d[:])
            bias_t = cpool.tile([P, 1], f32)
            nc.vector.memset(bias_t[:], -4.5)
            W_T0 = wts[:, 0:P]
            W_TRI = wts[:, P:2 * P]
            W_T3 = wts[:, 2 * P:3 * P]
            W_U = wts[:, 3 * P:4 * P]
            W_L = wts[:, 4 * P:5 * P]

            ms, xs, accs = [], [], []
            for i in range(n_imgs):
                m = ipool.tile([P, FI], bf16, tag=f"m{i}", name=f"m{i}")
                m3 = m.rearrange("p (b c) -> p b c", c=FB)
                # zero the pad columns (slots 1 and 514 of each block)
                nc.gpsimd.memset(m3[:, :, 1:FB - 1:FB - 3], 0)
                ms.append(m)
                xs.append(ipool.tile([P, FD], bf16, tag=f"x{i}", name=f"x{i}"))
                accs.append(apool.tile([P, NACC], f32, tag=f"a{i}", name=f"a{i}"))

            # laddered prefetch: keep ~2 images in flight so arrivals track
            # consumption order
            tok = cpool.tile([P, 2 * n_imgs], bf16)
            for i in range(n_imgs):
                m3 = ms[i].rearrange("p (b c) -> p b c", c=FB)
                if i >= 1:
                    nc.gpsimd.tensor_copy(tok[:, 2 * i:2 * i + 1],
                                          ms[i - 1][:, 2:3])
                nc.gpsimd.dma_start(
                    m3[:, :, 2:FB - 2],
                    y_d[i].rearrange("(b p) w -> p b w", p=P))
                if i >= 1:
                    nc.gpsimd.tensor_copy(tok[:, 2 * i + 1:2 * i + 2],
                                          xs[i - 1][:, 0:1])
                nc.gpsimd.dma_start(
                    xs[i].rearrange("p (b c) -> p b c", c=W),
                    x_d[i].rearrange("(b p) w -> p b w", p=P))

            # HAM warm-up: keep PE busy while the first loads land
            warm = pp.tile([P, NB, W], f32, tag="sp", name="warm")
            for _ in range(20):
                nc.tensor.matmul(warm[:, 0], wts[:, 0:P], wts[:, 0:4 * P],
                                 start=True, stop=True)

            def frontA(i):
                """t-add and the vertical matmuls (DVE t first, then PE)."""
                m = ms[i]
                m3 = m.rearrange("p (b c) -> p b c", c=FB)
                mc = m3[:, :, 2:FB - 2]

                t = work.tile([P, FI], bf16, tag="t", name=f"t{i}")
                nc.vector.tensor_add(t[:, 0:FI - 2], m[:, 0:FI - 2], m[:, 2:FI])
                t3 = t.rearrange("p (b c) -> p b c", c=FB)

                sp = pp.tile([P, NB, W], f32, tag="sp", name=f"sp{i}")

                def mm(b, wt, rhs, **kw):
                    nc.tensor.matmul(sp[:, b], wt, rhs, **kw)

                for b, wt in ((0, W_T0), (1, W_TRI), (2, W_TRI), (3, W_T3)):
                    mm(b, wt, t3[:, b, 1:FB - 3], start=True, stop=False)
                    mm(b, wt, mc[:, b], start=False, stop=False)
                for b in (0, 1, 2):
                    mm(b, W_U, t3[:, b + 1, 1:FB - 3], start=False, stop=False)
                    mm(b, W_U, mc[:, b + 1], start=False, stop=(b == 0))
                for b in (1, 2, 3):
                    mm(b, W_L, t3[:, b - 1, 1:FB - 3], start=False, stop=False)
                    mm(b, W_L, mc[:, b - 1], start=False, stop=True)
                return sp

            def frontB(i):
                """z-path on DVE, softplus (exp/ln) on ACT."""
                m, xb, ac = ms[i], xs[i], accs[i]
                m3 = m.rearrange("p (b c) -> p b c", c=FB)
                mc = m3[:, :, 2:FB - 2]

                m2 = work.tile([P, FD], bf16, tag="m2", name=f"m2{i}")
                nc.vector.tensor_scalar(
                    m2.rearrange("p (b c) -> p b c", c=W), mc, 0.5, None,
                    Alu.subtract)
                zh = work.tile([P, FD], bf16, tag="zh", name=f"zh{i}")
                nc.vector.tensor_mul(zh[:], m2[:], xb[:])
                eb = work.tile([P, FD], bf16, tag="eb", name=f"eb{i}")
                nc.scalar.activation(eb[:], zh[:], Act.Exp, scale=-2.0)
                lt = work.tile([P, FD], bf16, tag="lt", name=f"lt{i}")
                nc.scalar.activation(lt[:], eb[:], Act.Ln, bias=1.0,
                                     accum_out=ac[:, 0:1])
                return lt

            def back(i, sp, lt):
                """abs, band STTs, column fix, accumulator DMA-out.

                Alternates abs-path block count (4 vs 3) per image to
                balance ACT vs DVE.
                """
                ac = accs[i]
                ab_blocks = 4 if i % 2 == 0 else 3
                fa = ab_blocks * W
                spd = sp.rearrange("p b c -> p (b c)")
                lt3 = lt.rearrange("p (b c) -> p b c", c=W)

                ab = work.tile([P, FD], bf16, tag="ab", name=f"ab{i}")
                nc.scalar.activation(ab[:, 0:fa], spd[:, 0:fa], Act.Abs,
                                     bias=bias_t[:])
                u1 = work.tile([P, FD], bf16, tag="u1", name=f"u1{i}")
                nc.vector.scalar_tensor_tensor(
                    u1[:, 0:fa], ab[:, 0:fa], 4.05, lt[:, 0:fa],
                    Alu.is_le, Alu.mult, accum_out=ac[:, 1:2])

                if ab_blocks == 3:
                    u2 = work.tile([P, W], bf16, tag="u2", name=f"u2{i}")
                    nc.vector.scalar_tensor_tensor(
                        u2[:], sp[:, 3], 0.45, lt3[:, 3],
                        Alu.is_ge, Alu.mult, accum_out=ac[:, 2:3])
                    u3 = work.tile([P, W], bf16, tag="u3", name=f"u3{i}")
                    nc.vector.scalar_tensor_tensor(
                        u3[:], sp[:, 3], 8.55, lt3[:, 3],
                        Alu.is_ge, Alu.mult, accum_out=ac[:, 3:4])
                else:
                    nc.vector.memset(ac[:, 2:4], 0.0)

                ec = work.tile([P, 2 * NB], bf16, tag="ec", name=f"ec{i}")
                nc.vector.scalar_tensor_tensor(
                    ec.rearrange("p (b c) -> p b c", c=2),
                    sp[:, :, ::W - 1], 5.5, lt3[:, :, ::W - 1],
                    Alu.is_ge, Alu.mult, accum_out=ac[:, 4:5])

                nc.sync.dma_start(acc_d[:, i * NACC:(i + 1) * NACC], ac[:])

            sps, lts = {}, {}
            sps[0] = frontA(0)
            lts[0] = frontB(0)
            for i in range(1, n_imgs):
                sps[i] = frontA(i)
                back(i - 1, sps[i - 1], lts[i - 1])
                lts[i] = frontB(i)
            back(n_imgs - 1, sps[n_imgs - 1], lts[n_imgs - 1])

    nc.compile()
    return nc


def _get_nc(n_imgs):
    if n_imgs not in _CACHE:
        _CACHE[n_imgs] = _build(n_imgs)
    return _CACHE[n_imgs]


def _combine(acc, n_imgs):
    a = acc.reshape(P, n_imgs, NACC).astype(np.float64)
    # total = sum(l) + u1 + u2 - u3 - colfix
    return (a[:, :, 0].sum() + a[:, :, 1].sum() + a[:, :, 2].sum()
            - a[:, :, 3].sum() - a[:, :, 4].sum())


def kernel(x, y):
    from concourse import bass_utils

    n = x.shape[0]
    per = n // N_CORES
    nc = _get_nc(per)
    wts = _consts()
    x = np.ascontiguousarray(x, dtype=np.float32)
    y = np.ascontiguousarray(y, dtype=np.int32)
    in_maps = [
        {"x": x[c * per:(c + 1) * per], "y": y[c * per:(c + 1) * per],
         "wts": wts}
        for c in range(N_CORES)
    ]
    res = bass_utils.run_bass_kernel_spmd(nc, in_maps, core_ids=list(range(N_CORES)))
    total = 0.0
    for r in res.results:
        total += _combine(r["acc"], per)
    return np.float32(total / (n * H * W))


# revision 12
# speedup vs baseline: 1.0873x; 1.0006x over previous
"""BorderLoss Trainium2 kernel (v6 = v4 + select-path rebalance).

Reference (per element, then global mean over [64,512,512]):
    loss = softplus((1-2y)*x)   (stable BCE identity, y binary)
    m = (y > 0);  border = dilate3x3(m) - erode3x3(m)  (SAME, OOB ignored)
    w = 1 + border;  out = mean(loss * w)

Scheme (v4 core, validated elementwise-exact vs reference in numpy):
  * v = 3x3 box-count of m with OOB=0, computed as horizontal 3-tap then
    vertical 3-tap.  Rows 0/511 get an extra 1.5x scale (folded into the
    tridiagonal matmul weights), after which ONE uniform band test
    |v - 4.5| <= 4.05  (i.e. 1 <= v <= 8) is exact everywhere except
    columns 0/511, fixed by a single strided STT with threshold 5.5
    (which also handles the corners exactly).
  * Horizontal 3-tap: outer pair (left+right) via one DVE bf16 2x add on
    a padded layout [P, 4, 516] (pads zero); the center tap is folded
    into the vertical matmul by running every tridiag/U/L pass twice.
  * Vertical 3-tap: per 128-row block, tridiagonal matmul on PE with
    single-entry U/L matrices carrying the cross-block rows.  All 20
    matmuls per image land in ONE [P, 4, 512] PSUM tile (v6).
  * loss: zh = (m - 0.5) * x  (DVE TS+TT), then ACT Exp(scale=-2) and
    Ln(bias=1) = softplus((1-2m)x), with accum_out giving sum(l) free.
  * border-weighted sum (v6 rebalance): ALTERNATING per image --
    even images: ACT Abs(v-4.5) over all 4 blocks + one DVE STT
    (<=4.05)*l;  odd images: Abs over blocks 0-2 + STT, block 3 via two
    one-sided PSUM STTs (>=0.45 minus >=8.55)*l.  This equalizes ACT vs
    DVE load (v4 pinned 3/1 and left DVE the bottleneck).  One merged
    column-fix STT (>=5.5 at cols 0/511 of all 4 blocks) replaces v4's
    two.  Host combine:
      total = sum(l) + u1 + u2 - u3 - colfix
"""

import sys
import numpy as np

if "/opt/trn_rl_repo" not in sys.path:
    sys.path.insert(0, "/opt/trn_rl_repo")

# ---- pin exp/ln/abs/square to the single covering activation-table set ----
from concourse import hw_specs as _hw
import functools as _ft

if not getattr(_hw.get_activation_tables, "_borderloss_patched", False):
    _orig_tabs = getattr(_hw.get_activation_tables, "__wrapped__",
                         _hw.get_activation_tables)

    @_ft.cache
    def _patched_tabs(module_arch):
        from concourse import mybir as _mb
        A = _mb.ActivationFunctionType
        strip = {A.Exp, A.Ln, A.Abs, A.Square}
        out = {}
        for k, v in _orig_tabs(module_arch).items():
            out[k] = v if k == "natural_log_exp_and_others" else v - strip
        return out

    _patched_tabs._borderloss_patched = True
    _hw.get_activation_tables = _patched_tabs

H = W = 512
P = 128
NB = 4               # 128-row blocks per image
FB = 516             # padded block width (data at cols 2..513, zeros at 1, 514)
FI = NB * FB         # 2064 padded free cols per image
FD = NB * W          # 2048 dense free cols per image
NACC = 6
N_CORES = 8

_CACHE = {}


def _consts():
    import ml_dtypes
    bf = ml_dtypes.bfloat16
    tri = np.zeros((P, P), dtype=np.float64)
    for k in range(P):
        tri[k, max(0, k - 1):min(P, k + 2)] = 1.0
    t0 = tri.copy()
    t0[0:2, 0] = 1.5          # scale row 0 so the uniform band is exact
    t3 = tri.copy()
    t3[126:128, 127] = 1.5
    u = np.zeros((P, P), dtype=np.float64)
    u[0, 127] = 1.0           # next block's row 0 -> out row 127
    lm = np.zeros((P, P), dtype=np.float64)
    lm[127, 0] = 1.0          # prev block's row 127 -> out row 0
    wts = np.concatenate([t0, tri, t3, u, lm], axis=1).astype(bf)
    return wts


def _build(n_imgs):
    import concourse.bass as bass
    import concourse.bacc as bacc
    import concourse.tile as tile
    from concourse import mybir

    f32 = mybir.dt.float32
    bf16 = mybir.dt.bfloat16
    i32 = mybir.dt.int32
    Alu = mybir.AluOpType
    Act = mybir.ActivationFunctionType

    nc = bacc.Bacc(None, target_bir_lowering=False)
    x_d = nc.dram_tensor("x", [n_imgs, H, W], f32, kind="ExternalInput")
    y_d = nc.dram_tensor("y", [n_imgs, H, W], i32, kind="ExternalInput")
    w_d = nc.dram_tensor("wts", [P, 5 * P], bf16, kind="ExternalInput")
    acc_d = nc.dram_tensor("acc", [P, n_imgs * NACC], f32, kind="ExternalOutput")

    with tile.TileContext(nc) as tc:
        with (
            tc.tile_pool(name="consts", bufs=1) as cpool,
            tc.tile_pool(name="inputs", bufs=1) as ipool,
            tc.tile_pool(name="work", bufs=4) as work,
            tc.tile_pool(name="accp", bufs=1) as apool,
            tc.tile_pool(name="ps", bufs=2, space=bass.MemorySpace.PSUM) as pp,
        ):
            wts = cpool.tile([P, 5 * P], bf16)
            nc.sync.dma_start(wts[:], w_d[:])
            bias_t = cpool.tile([P, 1], f32)
            nc.vector.memset(bias_t[:], -4.5)
            W_T0 = wts[:, 0:P]
            W_TRI = wts[:, P:2 * P]
            W_T3 = wts[:, 2 * P:3 * P]
            W_U = wts[:, 3 * P:4 * P]
            W_L = wts[:, 4 * P:5 * P]

            ms, xs, accs = [], [], []
            for i in range(n_imgs):
                m = ipool.tile([P, FI], bf16, tag=f"m{i}", name=f"m{i}")
                m3 = m.rearrange("p (b c) -> p b c", c=FB)
                # zero the pad columns (slots 1 and 514 of each block)
                nc.gpsimd.memset(m3[:, :, 1:FB - 1:FB - 3], 0)
                ms.append(m)
                xs.append(ipool.tile([P, FD], bf16, tag=f"x{i}", name=f"x{i}"))
                accs.append(apool.tile([P, NACC], f32, tag=f"a{i}", name=f"a{i}"))

            # laddered prefetch, depth 2: desc-gen for image i gates on the
            # arrival of image i-2, keeping two transfers in flight per
            # stream (depth-1 serialized transfers back-to-back and left
            # the DMA engines ~10% idle)
            tok = cpool.tile([P, 2 * n_imgs], bf16)
            for i in range(n_imgs):
                m3 = ms[i].rearrange("p (b c) -> p b c", c=FB)
                if i >= 2:
                    nc.gpsimd.tensor_copy(tok[:, 2 * i:2 * i + 1],
                                          ms[i - 2][:, 2:3])
                nc.gpsimd.dma_start(
                    m3[:, :, 2:FB - 2],
                    y_d[i].rearrange("(b p) w -> p b w", p=P))
                if i >= 2:
                    nc.gpsimd.tensor_copy(tok[:, 2 * i + 1:2 * i + 2],
                                          xs[i - 2][:, 0:1])
                nc.gpsimd.dma_start(
                    xs[i].rearrange("p (b c) -> p b c", c=W),
                    x_d[i].rearrange("(b p) w -> p b w", p=P))

            # HAM warm-up: keep PE busy while the first loads land
            warm = pp.tile([P, NB, W], f32, tag="sp", name="warm")
            for _ in range(8):
                nc.tensor.matmul(warm[:, 0], wts[:, 0:P], wts[:, 0:4 * P],
                                 start=True, stop=True)

            def frontA(i):
                """t-add and the vertical matmuls (DVE t first, then PE)."""
                m = ms[i]
                m3 = m.rearrange("p (b c) -> p b c", c=FB)
                mc = m3[:, :, 2:FB - 2]

                t = work.tile([P, FI], bf16, tag="t", name=f"t{i}")
                nc.vector.tensor_add(t[:, 0:FI - 2], m[:, 0:FI - 2], m[:, 2:FI])
                t3 = t.rearrange("p (b c) -> p b c", c=FB)

                sp = pp.tile([P, NB, W], f32, tag="sp", name=f"sp{i}")

                def mm(b, wt, src, start=False, stop=False):
                    nc.tensor.matmul(sp[:, b], wt, t3[:, src, 1:FB - 3],
                                     start=start, stop=False)
                    nc.tensor.matmul(sp[:, b], wt, mc[:, src], start=False,
                                     stop=stop)

                # grouped per PSUM bank so each bank completes as early as
                # possible (lets the abs/STT chunks pipeline under the
                # remaining matmuls)
                mm(0, W_T0, 0, start=True)
                mm(0, W_U, 1, stop=True)
                mm(1, W_TRI, 1, start=True)
                mm(1, W_U, 2)
                mm(1, W_L, 0, stop=True)
                mm(2, W_TRI, 2, start=True)
                mm(2, W_U, 3)
                mm(2, W_L, 1, stop=True)
                mm(3, W_T3, 3, start=True)
                mm(3, W_L, 2, stop=True)
                return sp

            def frontB(i):
                """z-path on DVE, softplus (exp/ln) on ACT."""
                m, xb, ac = ms[i], xs[i], accs[i]
                m3 = m.rearrange("p (b c) -> p b c", c=FB)
                mc = m3[:, :, 2:FB - 2]

                m2 = work.tile([P, FD], bf16, tag="m2", name=f"m2{i}")
                nc.vector.tensor_scalar(
                    m2.rearrange("p (b c) -> p b c", c=W), mc, 0.5, None,
                    Alu.subtract)
                zh = work.tile([P, FD], bf16, tag="zh", name=f"zh{i}")
                nc.vector.tensor_mul(zh[:], m2[:], xb[:])
                eb = work.tile([P, FD], bf16, tag="eb", name=f"eb{i}")
                nc.scalar.activation(eb[:], zh[:], Act.Exp, scale=-2.0)
                lt = work.tile([P, FD], bf16, tag="lt", name=f"lt{i}")
                nc.scalar.activation(lt[:], eb[:], Act.Ln, bias=1.0,
                                     accum_out=ac[:, 0:1])
                return lt

            def back(i, sp, lt):
                """abs, band STTs, column fix, accumulator DMA-out.

                Alternates abs-path block count (4 vs 3) per image to
                balance ACT vs DVE.
                """
                ac = accs[i]
                ab_blocks = 4 if i % 2 == 0 else 3
                spd = sp.rearrange("p b c -> p (b c)")
                lt3 = lt.rearrange("p (b c) -> p b c", c=W)

                # abs-path in two chunks (blocks 0-1, then the rest) so the
                # first chunk pipelines under the remaining matmuls
                ha = 2 * W
                fa = ab_blocks * W
                ab = work.tile([P, FD], bf16, tag="ab", name=f"ab{i}")
                nc.scalar.activation(ab[:, 0:ha], spd[:, 0:ha], Act.Abs,
                                     bias=bias_t[:])
                u1 = work.tile([P, FD], bf16, tag="u1", name=f"u1{i}")
                nc.vector.scalar_tensor_tensor(
                    u1[:, 0:ha], ab[:, 0:ha], 4.05, lt[:, 0:ha],
                    Alu.is_le, Alu.mult, accum_out=ac[:, 1:2])
                nc.scalar.activation(ab[:, ha:fa], spd[:, ha:fa], Act.Abs,
                                     bias=bias_t[:])
                nc.vector.scalar_tensor_tensor(
                    u1[:, ha:fa], ab[:, ha:fa], 4.05, lt[:, ha:fa],
                    Alu.is_le, Alu.mult, accum_out=ac[:, 2:3])

                if ab_blocks == 3:
                    u2 = work.tile([P, W], bf16, tag="u2", name=f"u2{i}")
                    nc.vector.scalar_tensor_tensor(
                        u2[:], sp[:, 3], 0.45, lt3[:, 3],
                        Alu.is_ge, Alu.mult, accum_out=ac[:, 3:4])
                    u3 = work.tile([P, W], bf16, tag="u3", name=f"u3{i}")
                    nc.vector.scalar_tensor_tensor(
                        u3[:], sp[:, 3], 8.55, lt3[:, 3],
                        Alu.is_ge, Alu.mult, accum_out=ac[:, 4:5])
                else:
                    nc.vector.memset(ac[:, 3:5], 0.0)

                ec = work.tile([P, 2 * NB], bf16, tag="ec", name=f"ec{i}")
                nc.vector.scalar_tensor_tensor(
                    ec.rearrange("p (b c) -> p b c", c=2),
                    sp[:, :, ::W - 1], 5.5, lt3[:, :, ::W - 1],
                    Alu.is_ge, Alu.mult, accum_out=ac[:, 5:6])

                nc.sync.dma_start(acc_d[:, i * NACC:(i + 1) * NACC], ac[:])

            sps, lts = {}, {}
            sps[0] = frontA(0)
            lts[0] = frontB(0)
            for i in range(1, n_imgs):
                sps[i] = frontA(i)
                back(i - 1, sps[i - 1], lts[i - 1])
                lts[i] = frontB(i)
            back(n_imgs - 1, sps[n_imgs - 1], lts[n_imgs - 1])

    nc.compile()
    return nc


def _get_nc(n_imgs):
    if n_imgs not in _CACHE:
        _CACHE[n_imgs] = _build(n_imgs)
    return _CACHE[n_imgs]


def _combine(acc, n_imgs):
    a = acc.reshape(P, n_imgs, NACC).astype(np.float64)
    # total = sum(l) + u1a + u1b + u2 - u3 - colfix
    return (a[:, :, 0].sum() + a[:, :, 1].sum() + a[:, :, 2].sum()
            + a[:, :, 3].sum() - a[:, :, 4].sum() - a[:, :, 5].sum())


def kernel(x, y):
    from concourse import bass_utils

    n = x.shape[0]
    per = n // N_CORES
    nc = _get_nc(per)
    wts = _consts()
    x = np.ascontiguousarray(x, dtype=np.float32)
    y = np.ascontiguousarray(y, dtype=np.int32)
    in_maps = [
        {"x": x[c * per:(c + 1) * per], "y": y[c * per:(c + 1) * per],
         "wts": wts}
        for c in range(N_CORES)
    ]
    res = bass_utils.run_bass_kernel_spmd(nc, in_maps, core_ids=list(range(N_CORES)))
    total = 0.0
    for r in res.results:
        total += _combine(r["acc"], per)
    return np.float32(total / (n * H * W))


# revision 13
# speedup vs baseline: 1.1946x; 1.0987x over previous
"""BorderLoss Trainium2 kernel (v6 = v4 + select-path rebalance).

Reference (per element, then global mean over [64,512,512]):
    loss = softplus((1-2y)*x)   (stable BCE identity, y binary)
    m = (y > 0);  border = dilate3x3(m) - erode3x3(m)  (SAME, OOB ignored)
    w = 1 + border;  out = mean(loss * w)

Scheme (v4 core, validated elementwise-exact vs reference in numpy):
  * v = 3x3 box-count of m with OOB=0, computed as horizontal 3-tap then
    vertical 3-tap.  Rows 0/511 get an extra 1.5x scale (folded into the
    tridiagonal matmul weights), after which ONE uniform band test
    |v - 4.5| <= 4.05  (i.e. 1 <= v <= 8) is exact everywhere except
    columns 0/511, fixed by a single strided STT with threshold 5.5
    (which also handles the corners exactly).
  * Horizontal 3-tap: outer pair (left+right) via one DVE bf16 2x add on
    a padded layout [P, 4, 516] (pads zero); the center tap is folded
    into the vertical matmul by running every tridiag/U/L pass twice.
  * Vertical 3-tap: per 128-row block, tridiagonal matmul on PE with
    single-entry U/L matrices carrying the cross-block rows.  All 20
    matmuls per image land in ONE [P, 4, 512] PSUM tile (v6).
  * loss: zh = (m - 0.5) * x  (DVE TS+TT), then ACT Exp(scale=-2) and
    Ln(bias=1) = softplus((1-2m)x), with accum_out giving sum(l) free.
  * border-weighted sum (v6 rebalance): ALTERNATING per image --
    even images: ACT Abs(v-4.5) over all 4 blocks + one DVE STT
    (<=4.05)*l;  odd images: Abs over blocks 0-2 + STT, block 3 via two
    one-sided PSUM STTs (>=0.45 minus >=8.55)*l.  This equalizes ACT vs
    DVE load (v4 pinned 3/1 and left DVE the bottleneck).  One merged
    column-fix STT (>=5.5 at cols 0/511 of all 4 blocks) replaces v4's
    two.  Host combine:
      total = sum(l) + u1 + u2 - u3 - colfix
"""

import sys
import numpy as np

if "/opt/trn_rl_repo" not in sys.path:
    sys.path.insert(0, "/opt/trn_rl_repo")

# ---- pin exp/ln/abs/square to the single covering activation-table set ----
from concourse import hw_specs as _hw
import functools as _ft

if not getattr(_hw.get_activation_tables, "_borderloss_patched", False):
    _orig_tabs = getattr(_hw.get_activation_tables, "__wrapped__",
                         _hw.get_activation_tables)

    @_ft.cache
    def _patched_tabs(module_arch):
        from concourse import mybir as _mb
        A = _mb.ActivationFunctionType
        strip = {A.Exp, A.Ln, A.Abs, A.Square}
        out = {}
        for k, v in _orig_tabs(module_arch).items():
            out[k] = v if k == "natural_log_exp_and_others" else v - strip
        return out

    _patched_tabs._borderloss_patched = True
    _hw.get_activation_tables = _patched_tabs

H = W = 512
P = 128
NB = 4               # 128-row blocks per image
FB = 516             # padded block width (data at cols 2..513, zeros at 1, 514)
FI = NB * FB         # 2064 padded free cols per image
FD = NB * W          # 2048 dense free cols per image
NACC = 6
N_CORES = 8

_CACHE = {}


def _consts():
    import ml_dtypes
    bf = ml_dtypes.bfloat16
    tri = np.zeros((P, P), dtype=np.float64)
    for k in range(P):
        tri[k, max(0, k - 1):min(P, k + 2)] = 1.0
    t0 = tri.copy()
    t0[0:2, 0] = 1.5          # scale row 0 so the uniform band is exact
    t3 = tri.copy()
    t3[126:128, 127] = 1.5
    u = np.zeros((P, P), dtype=np.float64)
    u[0, 127] = 1.0           # next block's row 0 -> out row 127
    lm = np.zeros((P, P), dtype=np.float64)
    lm[127, 0] = 1.0          # prev block's row 127 -> out row 0
    wts = np.concatenate([t0, tri, t3, u, lm], axis=1).astype(bf)
    return wts


def _build(n_imgs):
    import concourse.bass as bass
    import concourse.bacc as bacc
    import concourse.tile as tile
    from concourse import mybir

    f32 = mybir.dt.float32
    bf16 = mybir.dt.bfloat16
    i32 = mybir.dt.int32
    Alu = mybir.AluOpType
    Act = mybir.ActivationFunctionType

    nc = bacc.Bacc(None, target_bir_lowering=False)
    x_d = nc.dram_tensor("x", [n_imgs, H, W], f32, kind="ExternalInput")
    y_d = nc.dram_tensor("y", [n_imgs, H, W], i32, kind="ExternalInput")
    w_d = nc.dram_tensor("wts", [P, 5 * P], bf16, kind="ExternalInput")
    acc_d = nc.dram_tensor("acc", [P, n_imgs * NACC], f32, kind="ExternalOutput")

    with tile.TileContext(nc) as tc:
        with (
            tc.tile_pool(name="consts", bufs=1) as cpool,
            tc.tile_pool(name="inputs", bufs=1) as ipool,
            tc.tile_pool(name="work", bufs=4) as work,
            tc.tile_pool(name="accp", bufs=1) as apool,
            tc.tile_pool(name="ps", bufs=2, space=bass.MemorySpace.PSUM) as pp,
        ):
            wts = cpool.tile([P, 5 * P], bf16)
            nc.sync.dma_start(wts[:], w_d[:])
            bias_t = cpool.tile([P, 1], f32)
            nc.vector.memset(bias_t[:], -4.5)
            W_T0 = wts[:, 0:P]
            W_TRI = wts[:, P:2 * P]
            W_T3 = wts[:, 2 * P:3 * P]
            W_U = wts[:, 3 * P:4 * P]
            W_L = wts[:, 4 * P:5 * P]

            ms, xs, accs = [], [], []
            for i in range(n_imgs):
                ms.append(ipool.tile([P, FI], bf16, tag=f"m{i}", name=f"m{i}"))
                xs.append(ipool.tile([P, FD], bf16, tag=f"x{i}", name=f"x{i}"))
                accs.append(apool.tile([P, NACC], f32, tag=f"a{i}", name=f"a{i}"))

            # laddered prefetch, depth 2: desc-gen for image i gates on the
            # arrival of image i-2, keeping two transfers in flight per
            # stream.  y before x (the t->matmul chain consumes y first);
            # image 0's DMAs issue before the pad memsets (disjoint columns)
            # so the first transfers start as early as possible.
            tok = cpool.tile([P, 2 * n_imgs], bf16)

            def prefetch(i):
                m3 = ms[i].rearrange("p (b c) -> p b c", c=FB)
                if i >= 2:
                    nc.gpsimd.tensor_copy(tok[:, 2 * i:2 * i + 1],
                                          ms[i - 2][:, 2:3])
                nc.gpsimd.dma_start(
                    m3[:, :, 2:FB - 2],
                    y_d[i].rearrange("(b p) w -> p b w", p=P))
                if i >= 2:
                    nc.gpsimd.tensor_copy(tok[:, 2 * i + 1:2 * i + 2],
                                          xs[i - 2][:, 0:1])
                nc.gpsimd.dma_start(
                    xs[i].rearrange("p (b c) -> p b c", c=W),
                    x_d[i].rearrange("(b p) w -> p b w", p=P))

            prefetch(0)
            for i in range(n_imgs):
                m3 = ms[i].rearrange("p (b c) -> p b c", c=FB)
                # zero the pad columns (slots 1 and 514 of each block)
                nc.gpsimd.memset(m3[:, :, 1:FB - 1:FB - 3], 0)
            for i in range(1, n_imgs):
                prefetch(i)

            # HAM warm-up: keep PE busy while the first loads land
            warm = pp.tile([P, NB, W], f32, tag="sp", name="warm")
            for _ in range(8):
                nc.tensor.matmul(warm[:, 0], wts[:, 0:P], wts[:, 0:4 * P],
                                 start=True, stop=True)

            def frontA(i):
                """t-add and the vertical matmuls (DVE t first, then PE)."""
                m = ms[i]
                m3 = m.rearrange("p (b c) -> p b c", c=FB)
                mc = m3[:, :, 2:FB - 2]

                t = work.tile([P, FI], bf16, tag="t", name=f"t{i}")
                nc.vector.tensor_add(t[:, 0:FI - 2], m[:, 0:FI - 2], m[:, 2:FI])
                t3 = t.rearrange("p (b c) -> p b c", c=FB)

                sp = pp.tile([P, NB, W], f32, tag="sp", name=f"sp{i}")

                def mm(b, wt, src, start=False, stop=False):
                    nc.tensor.matmul(sp[:, b], wt, t3[:, src, 1:FB - 3],
                                     start=start, stop=False)
                    nc.tensor.matmul(sp[:, b], wt, mc[:, src], start=False,
                                     stop=stop)

                # grouped per PSUM bank so each bank completes as early as
                # possible (lets the abs/STT chunks pipeline under the
                # remaining matmuls)
                mm(0, W_T0, 0, start=True)
                mm(0, W_U, 1, stop=True)
                mm(1, W_TRI, 1, start=True)
                mm(1, W_U, 2)
                mm(1, W_L, 0, stop=True)
                mm(2, W_TRI, 2, start=True)
                mm(2, W_U, 3)
                mm(2, W_L, 1, stop=True)
                mm(3, W_T3, 3, start=True)
                mm(3, W_L, 2, stop=True)
                return sp

            def frontB(i):
                """z-path on DVE, softplus (exp/ln) on ACT."""
                m, xb, ac = ms[i], xs[i], accs[i]
                m3 = m.rearrange("p (b c) -> p b c", c=FB)
                mc = m3[:, :, 2:FB - 2]

                m2 = work.tile([P, FD], bf16, tag="m2", name=f"m2{i}")
                nc.vector.tensor_scalar(
                    m2.rearrange("p (b c) -> p b c", c=W), mc, 0.5, None,
                    Alu.subtract)
                zh = work.tile([P, FD], bf16, tag="zh", name=f"zh{i}")
                nc.vector.tensor_mul(zh[:], m2[:], xb[:])
                eb = work.tile([P, FD], bf16, tag="eb", name=f"eb{i}")
                nc.scalar.activation(eb[:], zh[:], Act.Exp, scale=-2.0)
                lt = work.tile([P, FD], bf16, tag="lt", name=f"lt{i}")
                nc.scalar.activation(lt[:], eb[:], Act.Ln, bias=1.0,
                                     accum_out=ac[:, 0:1])
                return lt

            def back(i, sp, lt):
                """abs, band STTs, column fix, accumulator DMA-out.

                Alternates abs-path block count (4 vs 3) per image to
                balance ACT vs DVE.
                """
                ac = accs[i]
                ab_blocks = 4 if i % 2 == 0 else 3
                spd = sp.rearrange("p b c -> p (b c)")
                lt3 = lt.rearrange("p (b c) -> p b c", c=W)

                # abs-path in two chunks (blocks 0-1, then the rest) so the
                # first chunk pipelines under the remaining matmuls
                ha = 2 * W
                fa = ab_blocks * W
                ab = work.tile([P, FD], bf16, tag="ab", name=f"ab{i}")
                nc.scalar.activation(ab[:, 0:ha], spd[:, 0:ha], Act.Abs,
                                     bias=bias_t[:])
                u1 = work.tile([P, FD], bf16, tag="u1", name=f"u1{i}")
                nc.vector.scalar_tensor_tensor(
                    u1[:, 0:ha], ab[:, 0:ha], 4.05, lt[:, 0:ha],
                    Alu.is_le, Alu.mult, accum_out=ac[:, 1:2])
                nc.scalar.activation(ab[:, ha:fa], spd[:, ha:fa], Act.Abs,
                                     bias=bias_t[:])
                nc.vector.scalar_tensor_tensor(
                    u1[:, ha:fa], ab[:, ha:fa], 4.05, lt[:, ha:fa],
                    Alu.is_le, Alu.mult, accum_out=ac[:, 2:3])

                if ab_blocks == 3:
                    u2 = work.tile([P, W], bf16, tag="u2", name=f"u2{i}")
                    nc.vector.scalar_tensor_tensor(
                        u2[:], sp[:, 3], 0.45, lt3[:, 3],
                        Alu.is_ge, Alu.mult, accum_out=ac[:, 3:4])
                    u3 = work.tile([P, W], bf16, tag="u3", name=f"u3{i}")
                    nc.vector.scalar_tensor_tensor(
                        u3[:], sp[:, 3], 8.55, lt3[:, 3],
                        Alu.is_ge, Alu.mult, accum_out=ac[:, 4:5])
                else:
                    nc.vector.memset(ac[:, 3:5], 0.0)

                ec = work.tile([P, 2 * NB], bf16, tag="ec", name=f"ec{i}")
                nc.vector.scalar_tensor_tensor(
                    ec.rearrange("p (b c) -> p b c", c=2),
                    sp[:, :, ::W - 1], 5.5, lt3[:, :, ::W - 1],
                    Alu.is_ge, Alu.mult, accum_out=ac[:, 5:6])

                nc.sync.dma_start(acc_d[:, i * NACC:(i + 1) * NACC], ac[:])

            sps, lts = {}, {}
            sps[0] = frontA(0)
            lts[0] = frontB(0)
            for i in range(1, n_imgs):
                sps[i] = frontA(i)
                back(i - 1, sps[i - 1], lts[i - 1])
                lts[i] = frontB(i)
            back(n_imgs - 1, sps[n_imgs - 1], lts[n_imgs - 1])

    nc.compile()
    return nc


def _get_nc(n_imgs):
    if n_imgs not in _CACHE:
        _CACHE[n_imgs] = _build(n_imgs)
    return _CACHE[n_imgs]


def _combine(acc, n_imgs):
    a = acc.reshape(P, n_imgs, NACC).astype(np.float64)
    # total = sum(l) + u1a + u1b + u2 - u3 - colfix
    return (a[:, :, 0].sum() + a[:, :, 1].sum() + a[:, :, 2].sum()
            + a[:, :, 3].sum() - a[:, :, 4].sum() - a[:, :, 5].sum())


def kernel(x, y):
    from concourse import bass_utils

    n = x.shape[0]
    per = n // N_CORES
    nc = _get_nc(per)
    wts = _consts()
    x = np.ascontiguousarray(x, dtype=np.float32)
    y = np.ascontiguousarray(y, dtype=np.int32)
    in_maps = [
        {"x": x[c * per:(c + 1) * per], "y": y[c * per:(c + 1) * per],
         "wts": wts}
        for c in range(N_CORES)
    ]
    res = bass_utils.run_bass_kernel_spmd(nc, in_maps, core_ids=list(range(N_CORES)))
    total = 0.0
    for r in res.results:
        total += _combine(r["acc"], per)
    return np.float32(total / (n * H * W))


# revision 14
# speedup vs baseline: 1.1998x; 1.0043x over previous
"""BorderLoss Trainium2 kernel (v6 = v4 + select-path rebalance).

Reference (per element, then global mean over [64,512,512]):
    loss = softplus((1-2y)*x)   (stable BCE identity, y binary)
    m = (y > 0);  border = dilate3x3(m) - erode3x3(m)  (SAME, OOB ignored)
    w = 1 + border;  out = mean(loss * w)

Scheme (v4 core, validated elementwise-exact vs reference in numpy):
  * v = 3x3 box-count of m with OOB=0, computed as horizontal 3-tap then
    vertical 3-tap.  Rows 0/511 get an extra 1.5x scale (folded into the
    tridiagonal matmul weights), after which ONE uniform band test
    |v - 4.5| <= 4.05  (i.e. 1 <= v <= 8) is exact everywhere except
    columns 0/511, fixed by a single strided STT with threshold 5.5
    (which also handles the corners exactly).
  * Horizontal 3-tap: outer pair (left+right) via one DVE bf16 2x add on
    a padded layout [P, 4, 516] (pads zero); the center tap is folded
    into the vertical matmul by running every tridiag/U/L pass twice.
  * Vertical 3-tap: per 128-row block, tridiagonal matmul on PE with
    single-entry U/L matrices carrying the cross-block rows.  All 20
    matmuls per image land in ONE [P, 4, 512] PSUM tile (v6).
  * loss: zh = (m - 0.5) * x  (DVE TS+TT), then ACT Exp(scale=-2) and
    Ln(bias=1) = softplus((1-2m)x), with accum_out giving sum(l) free.
  * border-weighted sum (v6 rebalance): ALTERNATING per image --
    even images: ACT Abs(v-4.5) over all 4 blocks + one DVE STT
    (<=4.05)*l;  odd images: Abs over blocks 0-2 + STT, block 3 via two
    one-sided PSUM STTs (>=0.45 minus >=8.55)*l.  This equalizes ACT vs
    DVE load (v4 pinned 3/1 and left DVE the bottleneck).  One merged
    column-fix STT (>=5.5 at cols 0/511 of all 4 blocks) replaces v4's
    two.  Host combine:
      total = sum(l) + u1 + u2 - u3 - colfix
"""

import sys
import numpy as np

if "/opt/trn_rl_repo" not in sys.path:
    sys.path.insert(0, "/opt/trn_rl_repo")

# ---- pin exp/ln/abs/square to the single covering activation-table set ----
from concourse import hw_specs as _hw
import functools as _ft

if not getattr(_hw.get_activation_tables, "_borderloss_patched", False):
    _orig_tabs = getattr(_hw.get_activation_tables, "__wrapped__",
                         _hw.get_activation_tables)

    @_ft.cache
    def _patched_tabs(module_arch):
        from concourse import mybir as _mb
        A = _mb.ActivationFunctionType
        strip = {A.Exp, A.Ln, A.Abs, A.Square}
        out = {}
        for k, v in _orig_tabs(module_arch).items():
            out[k] = v if k == "natural_log_exp_and_others" else v - strip
        return out

    _patched_tabs._borderloss_patched = True
    _hw.get_activation_tables = _patched_tabs

H = W = 512
P = 128
NB = 4               # 128-row blocks per image
FB = 516             # padded block width (data at cols 2..513, zeros at 1, 514)
FI = NB * FB         # 2064 padded free cols per image
FD = NB * W          # 2048 dense free cols per image
NACC = 6
N_CORES = 8

_CACHE = {}


def _consts():
    import ml_dtypes
    bf = ml_dtypes.bfloat16
    tri = np.zeros((P, P), dtype=np.float64)
    for k in range(P):
        tri[k, max(0, k - 1):min(P, k + 2)] = 1.0
    t0 = tri.copy()
    t0[0:2, 0] = 1.5          # scale row 0 so the uniform band is exact
    t3 = tri.copy()
    t3[126:128, 127] = 1.5
    u = np.zeros((P, P), dtype=np.float64)
    u[0, 127] = 1.0           # next block's row 0 -> out row 127
    lm = np.zeros((P, P), dtype=np.float64)
    lm[127, 0] = 1.0          # prev block's row 127 -> out row 0
    wts = np.concatenate([t0, tri, t3, u, lm], axis=1).astype(bf)
    return wts


def _build(n_imgs):
    import concourse.bass as bass
    import concourse.bacc as bacc
    import concourse.tile as tile
    from concourse import mybir

    f32 = mybir.dt.float32
    bf16 = mybir.dt.bfloat16
    i32 = mybir.dt.int32
    Alu = mybir.AluOpType
    Act = mybir.ActivationFunctionType

    nc = bacc.Bacc(None, target_bir_lowering=False)
    x_d = nc.dram_tensor("x", [n_imgs, H, W], f32, kind="ExternalInput")
    y_d = nc.dram_tensor("y", [n_imgs, H, W], i32, kind="ExternalInput")
    w_d = nc.dram_tensor("wts", [P, 5 * P], bf16, kind="ExternalInput")
    acc_d = nc.dram_tensor("acc", [P, n_imgs * NACC], f32, kind="ExternalOutput")

    with tile.TileContext(nc) as tc:
        with (
            tc.tile_pool(name="consts", bufs=1) as cpool,
            tc.tile_pool(name="inputs", bufs=1) as ipool,
            tc.tile_pool(name="work", bufs=4) as work,
            tc.tile_pool(name="accp", bufs=1) as apool,
            tc.tile_pool(name="ps", bufs=2, space=bass.MemorySpace.PSUM) as pp,
        ):
            wts = cpool.tile([P, 5 * P], bf16)
            nc.sync.dma_start(wts[:], w_d[:])
            bias_t = cpool.tile([P, 1], f32)
            nc.vector.memset(bias_t[:], -4.5)
            W_T0 = wts[:, 0:P]
            W_TRI = wts[:, P:2 * P]
            W_T3 = wts[:, 2 * P:3 * P]
            W_U = wts[:, 3 * P:4 * P]
            W_L = wts[:, 4 * P:5 * P]

            ms, xs, accs = [], [], []
            for i in range(n_imgs):
                ms.append(ipool.tile([P, FI], bf16, tag=f"m{i}", name=f"m{i}"))
                xs.append(ipool.tile([P, FD], bf16, tag=f"x{i}", name=f"x{i}"))
                accs.append(apool.tile([P, NACC], f32, tag=f"a{i}", name=f"a{i}"))

            # laddered prefetch, depth 2: desc-gen for image i gates on the
            # arrival of image i-2, keeping two transfers in flight per
            # stream.  y before x (the t->matmul chain consumes y first);
            # image 0's DMAs issue before the pad memsets (disjoint columns)
            # so the first transfers start as early as possible.
            tok = cpool.tile([P, 2 * n_imgs], bf16)

            def prefetch(i):
                m3 = ms[i].rearrange("p (b c) -> p b c", c=FB)
                if i >= 2:
                    nc.gpsimd.tensor_copy(tok[:, 2 * i:2 * i + 1],
                                          ms[i - 2][:, 2:3])
                nc.gpsimd.dma_start(
                    m3[:, :, 2:FB - 2],
                    y_d[i].rearrange("(b p) w -> p b w", p=P))
                if i >= 2:
                    nc.gpsimd.tensor_copy(tok[:, 2 * i + 1:2 * i + 2],
                                          xs[i - 2][:, 0:1])
                nc.gpsimd.dma_start(
                    xs[i].rearrange("p (b c) -> p b c", c=W),
                    x_d[i].rearrange("(b p) w -> p b w", p=P))

            prefetch(0)
            for i in range(n_imgs):
                m3 = ms[i].rearrange("p (b c) -> p b c", c=FB)
                # zero the pad columns (slots 1 and 514 of each block)
                nc.gpsimd.memset(m3[:, :, 1:FB - 1:FB - 3], 0)
            for i in range(1, n_imgs):
                prefetch(i)

            # HAM warm-up: keep PE busy while the first loads land
            warm = pp.tile([P, NB, W], f32, tag="sp", name="warm")
            for _ in range(8):
                nc.tensor.matmul(warm[:, 0], wts[:, 0:P], wts[:, 0:4 * P],
                                 start=True, stop=True)

            def frontA(i):
                """t-add and the vertical matmuls (DVE t first, then PE)."""
                m = ms[i]
                m3 = m.rearrange("p (b c) -> p b c", c=FB)
                mc = m3[:, :, 2:FB - 2]

                t = work.tile([P, FI], bf16, tag="t", name=f"t{i}")
                nc.vector.tensor_add(t[:, 0:FI - 2], m[:, 0:FI - 2], m[:, 2:FI])
                t3 = t.rearrange("p (b c) -> p b c", c=FB)

                sp = pp.tile([P, NB, W], f32, tag="sp", name=f"sp{i}")

                def mm(b, wt, src, start=False, stop=False):
                    nc.tensor.matmul(sp[:, b], wt, t3[:, src, 1:FB - 3],
                                     start=start, stop=False)
                    nc.tensor.matmul(sp[:, b], wt, mc[:, src], start=False,
                                     stop=stop)

                # grouped per PSUM bank so each bank completes as early as
                # possible (lets the abs/STT chunks pipeline under the
                # remaining matmuls)
                mm(0, W_T0, 0, start=True)
                mm(0, W_U, 1, stop=True)
                mm(1, W_TRI, 1, start=True)
                mm(1, W_U, 2)
                mm(1, W_L, 0, stop=True)
                mm(2, W_TRI, 2, start=True)
                mm(2, W_U, 3)
                mm(2, W_L, 1, stop=True)
                mm(3, W_T3, 3, start=True)
                mm(3, W_L, 2, stop=True)
                return sp

            def frontB(i):
                """z-path on DVE, softplus (exp/ln) on ACT."""
                m, xb, ac = ms[i], xs[i], accs[i]
                m3 = m.rearrange("p (b c) -> p b c", c=FB)
                mc = m3[:, :, 2:FB - 2]

                m2 = work.tile([P, FD], bf16, tag="m2", name=f"m2{i}")
                nc.vector.tensor_scalar(
                    m2.rearrange("p (b c) -> p b c", c=W), mc, 0.5, None,
                    Alu.subtract)
                zh = work.tile([P, FD], bf16, tag="zh", name=f"zh{i}")
                nc.vector.tensor_mul(zh[:], m2[:], xb[:])
                eb = work.tile([P, FD], bf16, tag="eb", name=f"eb{i}")
                nc.scalar.activation(eb[:], zh[:], Act.Exp, scale=-2.0)
                lt = work.tile([P, FD], bf16, tag="lt", name=f"lt{i}")
                nc.scalar.activation(lt[:], eb[:], Act.Ln, bias=1.0,
                                     accum_out=ac[:, 0:1])
                return lt

            def back(i, sp, lt):
                """abs, band STTs, column fix, accumulator DMA-out.

                Alternates abs-path block count (4 vs 3) per image to
                balance ACT vs DVE.
                """
                ac = accs[i]
                ab_blocks = 4
                spd = sp.rearrange("p b c -> p (b c)")
                lt3 = lt.rearrange("p (b c) -> p b c", c=W)

                # abs-path in two chunks (blocks 0-1, then the rest) so the
                # first chunk pipelines under the remaining matmuls
                ha = 2 * W
                fa = ab_blocks * W
                ab = work.tile([P, FD], bf16, tag="ab", name=f"ab{i}")
                nc.scalar.activation(ab[:, 0:ha], spd[:, 0:ha], Act.Abs,
                                     bias=bias_t[:])
                u1 = work.tile([P, FD], bf16, tag="u1", name=f"u1{i}")
                nc.vector.scalar_tensor_tensor(
                    u1[:, 0:ha], ab[:, 0:ha], 4.05, lt[:, 0:ha],
                    Alu.is_le, Alu.mult, accum_out=ac[:, 1:2])
                nc.scalar.activation(ab[:, ha:fa], spd[:, ha:fa], Act.Abs,
                                     bias=bias_t[:])
                nc.vector.scalar_tensor_tensor(
                    u1[:, ha:fa], ab[:, ha:fa], 4.05, lt[:, ha:fa],
                    Alu.is_le, Alu.mult, accum_out=ac[:, 2:3])

                if ab_blocks == 3:
                    u2 = work.tile([P, W], bf16, tag="u2", name=f"u2{i}")
                    nc.vector.scalar_tensor_tensor(
                        u2[:], sp[:, 3], 0.45, lt3[:, 3],
                        Alu.is_ge, Alu.mult, accum_out=ac[:, 3:4])
                    u3 = work.tile([P, W], bf16, tag="u3", name=f"u3{i}")
                    nc.vector.scalar_tensor_tensor(
                        u3[:], sp[:, 3], 8.55, lt3[:, 3],
                        Alu.is_ge, Alu.mult, accum_out=ac[:, 4:5])
                else:
                    nc.vector.memset(ac[:, 3:5], 0.0)

                ec = work.tile([P, 2 * NB], bf16, tag="ec", name=f"ec{i}")
                nc.vector.scalar_tensor_tensor(
                    ec.rearrange("p (b c) -> p b c", c=2),
                    sp[:, :, ::W - 1], 5.5, lt3[:, :, ::W - 1],
                    Alu.is_ge, Alu.mult, accum_out=ac[:, 5:6])

                nc.sync.dma_start(acc_d[:, i * NACC:(i + 1) * NACC], ac[:])

            sps, lts = {}, {}
            sps[0] = frontA(0)
            lts[0] = frontB(0)
            for i in range(1, n_imgs):
                sps[i] = frontA(i)
                back(i - 1, sps[i - 1], lts[i - 1])
                lts[i] = frontB(i)
            back(n_imgs - 1, sps[n_imgs - 1], lts[n_imgs - 1])

    nc.compile()
    return nc


def _get_nc(n_imgs):
    if n_imgs not in _CACHE:
        _CACHE[n_imgs] = _build(n_imgs)
    return _CACHE[n_imgs]


def _combine(acc, n_imgs):
    a = acc.reshape(P, n_imgs, NACC).astype(np.float64)
    # total = sum(l) + u1a + u1b + u2 - u3 - colfix
    return (a[:, :, 0].sum() + a[:, :, 1].sum() + a[:, :, 2].sum()
            + a[:, :, 3].sum() - a[:, :, 4].sum() - a[:, :, 5].sum())


def kernel(x, y):
    from concourse import bass_utils

    n = x.shape[0]
    per = n // N_CORES
    nc = _get_nc(per)
    wts = _consts()
    x = np.ascontiguousarray(x, dtype=np.float32)
    y = np.ascontiguousarray(y, dtype=np.int32)
    in_maps = [
        {"x": x[c * per:(c + 1) * per], "y": y[c * per:(c + 1) * per],
         "wts": wts}
        for c in range(N_CORES)
    ]
    res = bass_utils.run_bass_kernel_spmd(nc, in_maps, core_ids=list(range(N_CORES)))
    total = 0.0
    for r in res.results:
        total += _combine(r["acc"], per)
    return np.float32(total / (n * H * W))
